# revision 2
# baseline (speedup 1.0000x reference)
"""DiskKinematics v2: A=2 arithmetic fine radix, CB=26 coarse one-hot.

Differences vs baseline:
  - Radix idx = 2c + u with u computed ARITHMETICALLY (u = ft - 2*ck + 192,
    exact in bf16) instead of a 3-wide fine one-hot; the mass splits into
    mohf = [m*(1-u), m*u] which becomes the f=0 column pair of the
    stationary operand.
  - Coarse one-hot built with 26 tensor_scalar is_equal ops (4x DVE mode)
    instead of wide tensor_tensor vs an iota tile (2x).
  - PE operands swapped: stationary = E [128, 14] (LDW 14 cols), moving =
    OC [128, 26].  psum [14, 26] accumulated over all groups; optional
    PSUM-bank alternation and column-quadrant rotation.
  - Full-tile (G-wide) DVE ops, tile-level double buffering, no sub-batches.
  - Magic-number chain (ft, ck, v) runs on the ACT engine.

Output hist [n_acc*32 or 14, 26] per core; host reduces quadrant/bank
copies, reorders (a,f,c) -> bins, drops virtual bins 50/51, and computes
the means/sigmas in f64.
"""

import numpy as np
import ml_dtypes

import concourse.bacc as bacc
import concourse.mybir as mybir
from concourse.tile import TileContext
from concourse.bass_utils import run_bass_kernel_spmd

P = 128
N_CORES = 8
CB = 26            # coarse bins; virtual bins = 2*26 = 52, drop 50/51
F = 7              # m + 6 weighted moments
G = 625            # particles per partition per tile
NPLANES = 11       # x y vx vy m vz | vz2 sp sp2 tp tp2
KEY0 = 192.0

f32 = mybir.dt.float32
bf16 = mybir.dt.bfloat16

_CACHE = {}


def _act_raw(nc, out, in_, func, scale=1.0, bias=0.0):
    """InstActivation without the bass wrapper's Rsqrt ban (baseline trick)."""
    import concourse.bass as bass

    eng = nc.scalar
    bias_ap = nc.const_aps.scalar_like(bias, in_)
    inputs = [eng.lower_ap(in_)]
    for arg in (bias_ap, scale, 0.0):
        if isinstance(arg, bass.AP):
            inputs.append(eng.lower_ap(arg))
        else:
            inputs.append(mybir.ImmediateValue(dtype=f32, value=float(arg)))
    return eng.add_instruction(
        mybir.InstActivation(
            name=nc.get_next_instruction_name(),
            func=func,
            ins=inputs,
            outs=[eng.lower_ap(out)],
        )
    )


def _build(n_tiles: int, reps: int = 1, ablate: str = "", n_quads: int = 1,
           n_banks: int = 1):
    nc = bacc.Bacc(None, target_bir_lowering=False, debug=False)
    data = nc.dram_tensor("data", [n_tiles, P, 6 * G], bf16, kind="ExternalInput")
    hist = nc.dram_tensor("hist", [n_banks, P, CB], f32, kind="ExternalOutput")

    AO = mybir.AluOpType
    AF = mybir.ActivationFunctionType

    n_mm = n_tiles * G
    n_acc = n_quads * n_banks  # independent accumulators

    with TileContext(nc) as tc:
        with (
            tc.tile_pool(name="io", bufs=3) as iop,
            tc.tile_pool(name="fp", bufs=2) as fpp,
            tc.tile_pool(name="ar", bufs=2) as arp,
            tc.tile_pool(name="eo", bufs=2) as eop,
            tc.tile_pool(name="cst", bufs=1) as cst,
            tc.tile_pool(name="ps", bufs=1, space="PSUM") as psp,
        ):
            pss = [psp.tile([P, CB], f32, name=f"ps{b}") for b in range(n_banks)]

            def body():
                gi = 0
                for t in range(n_tiles):
                    C = iop.tile([P, NPLANES * G], bf16, tag="C")
                    nc.sync.dma_start(out=C[:, 0 : 6 * G], in_=data[t])

                    def pl(i, w=1):
                        return C[:, i * G : (i + w) * G]

                    x2 = pl(0, 2)      # x | y
                    vxy = pl(2, 2)     # vx | vy
                    m = pl(4)
                    vz = pl(5)

                    if ablate == "dma":
                        continue

                    xy2 = fpp.tile([P, 2 * G], bf16, tag="xy2")
                    rsq = fpp.tile([P, G], bf16, tag="rsq")
                    t5 = arp.tile([P, G], bf16, tag="t5")
                    i5 = arp.tile([P, G], bf16, tag="i5")
                    ft = arp.tile([P, G], bf16, tag="ft")
                    ck = arp.tile([P, G], bf16, tag="ck")
                    vt = arp.tile([P, G], bf16, tag="vt")
                    u = arp.tile([P, G], bf16, tag="u")
                    aabb = arp.tile([P, 2 * G], bf16, tag="aabb")
                    ccdd = arp.tile([P, 2 * G], bf16, tag="ccdd")
                    st = arp.tile([P, 2 * G], bf16, tag="st")
                    E = eop.tile([P, 2 * F * G], bf16, tag="E")
                    OC = eop.tile([P, CB * G], bf16, tag="OC")

                    # --- keys ------------------------------------------
                    if ablate == "mmonly":
                        pass
                    else:
                        _compute(C, xy2, rsq, t5, i5, ft, ck, vt, u, aabb,
                                 ccdd, st, E, OC, x2, vxy, m, nc, AO, AF)
                    if ablate in ("nomm", "mmonly2"):
                        continue
                    if False:
                        nc.scalar.activation(xy2[:], x2, AF.Square)
                    nc.vector.tensor_tensor(
                        out=rsq[:], in0=xy2[:, 0:G], in1=xy2[:, G : 2 * G],
                        op=AO.add,
                    )
                    nc.scalar.activation(t5[:], rsq[:], AF.Sqrt, scale=25.0)
                    _act_raw(nc, i5[:], rsq[:], AF.Rsqrt, scale=25.0)
                    # ft = 192 + floor(t5); ck = 192 + c; u = idx - 2c
                    # bias 191.5 + 2^-9: exact floor for all bf16 t5 in [0,64),
                    # no RNE ties (ties at integer t5 would misbin ~5%)
                    nc.scalar.activation(
                        ft[:], t5[:], AF.Copy, bias=KEY0 - 0.5 + 2.0**-9, scale=1.0
                    )
                    nc.scalar.activation(
                        ck[:], ft[:], AF.Copy, bias=95.75, scale=0.5
                    )
                    nc.scalar.activation(
                        vt[:], ck[:], AF.Copy, bias=KEY0, scale=-2.0
                    )
                    nc.vector.tensor_tensor(
                        out=u[:], in0=ft[:], in1=vt[:], op=AO.add
                    )

                    # --- velocity moments ------------------------------
                    nc.vector.tensor_tensor(out=aabb[:], in0=x2, in1=vxy, op=AO.mult)
                    yx = x2.rearrange("p (c g) -> p c g", c=2)[:, ::-1, :]
                    nc.vector.tensor_tensor(
                        out=ccdd[:].rearrange("p (c g) -> p c g", c=2),
                        in0=yx,
                        in1=vxy.rearrange("p (c g) -> p c g", c=2),
                        op=AO.mult,
                    )
                    nc.vector.tensor_tensor(
                        out=st[:, 0:G], in0=aabb[:, 0:G], in1=aabb[:, G : 2 * G],
                        op=AO.add,
                    )
                    nc.vector.tensor_tensor(
                        out=st[:, G : 2 * G], in0=ccdd[:, 0:G],
                        in1=ccdd[:, G : 2 * G], op=AO.subtract,
                    )
                    # sp -> plane 7, tp -> plane 9 (one strided op)
                    sptp = (
                        C[:, 7 * G : 11 * G]
                        .rearrange("p (c g) -> p c g", c=4)[:, ::2, :]
                    )
                    i5b = i5[:].unsqueeze(1).broadcast_to([P, 2, G])
                    nc.vector.tensor_tensor(
                        out=sptp,
                        in0=st[:].rearrange("p (c g) -> p c g", c=2),
                        in1=i5b,
                        op=AO.mult,
                    )
                    # squares: planes {5,7,9} -> {6,8,10} (one ACT op)
                    sq_in = (
                        C[:, 5 * G : 11 * G]
                        .rearrange("p (c g) -> p c g", c=6)[:, ::2, :]
                    )
                    sq_out = (
                        C[:, 5 * G : 11 * G]
                        .rearrange("p (c g) -> p c g", c=6)[:, 1::2, :]
                    )
                    nc.scalar.activation(sq_out, sq_in, AF.Square)

                    # --- E: [mohf0, f1..f6(a0), mohf1, f1..f6(a1)] -----
                    nc.vector.tensor_tensor(
                        out=E[:, F * G : (F + 1) * G], in0=m, in1=u[:], op=AO.mult
                    )
                    nc.vector.tensor_tensor(
                        out=E[:, 0:G], in0=m, in1=E[:, F * G : (F + 1) * G],
                        op=AO.subtract,
                    )
                    E4 = E[:].rearrange("p (a f g) -> p a f g", a=2, f=F)
                    mo_b = (
                        E[:].rearrange("p (a g) -> p a g", a=2 * F)
                        .rearrange("p (a f) g -> p a f g", a=2)[:, :, 0:1, :]
                        .broadcast_to([P, 2, F - 1, G])
                    )
                    feats_b = (
                        C[:, 5 * G : 11 * G]
                        .rearrange("p (f g) -> p f g", f=F - 1)
                        .unsqueeze(1)
                        .broadcast_to([P, 2, F - 1, G])
                    )
                    nc.vector.tensor_tensor(
                        out=E4[:, :, 1:F, :], in0=mo_b, in1=feats_b, op=AO.mult
                    )

                    # --- coarse one-hot --------------------------------
                    for c in range(CB):
                        nc.vector.tensor_scalar(
                            OC[:, c * G : (c + 1) * G], ck[:], KEY0 + c, None,
                            AO.is_equal,
                        )

                    if ablate == "nomm":
                        continue

                    # --- matmuls ---------------------------------------
                    E3 = E[:].rearrange("p (x g) -> p x g", x=2 * F)
                    OC3 = OC[:].rearrange("p (x g) -> p x g", x=CB)
                    for g in range(G):
                        acc = gi % n_acc
                        b, q = divmod(acc, n_quads)
                        nc.tensor.matmul(
                            out=pss[b][32 * q : 32 * q + 2 * F, 0:CB],
                            lhsT=E3[:, :, g],
                            rhs=OC3[:, :, g],
                            start=(gi < n_acc),
                            stop=(gi >= n_mm - n_acc),
                            skip_group_check=True,
                        )
                        gi += 1

            if reps == 1:
                body()
            else:
                with tc.For_i(0, reps):
                    body()

            out_sb = cst.tile([P, n_banks * CB], f32)
            if ablate:
                nc.vector.memset(out_sb[:], 0.0)
            else:
                for b in range(n_banks):
                    nc.vector.tensor_copy(
                        out=out_sb[:, b * CB : (b + 1) * CB], in_=pss[b][:]
                    )
            for b in range(n_banks):
                nc.sync.dma_start(
                    out=hist[b], in_=out_sb[:, b * CB : (b + 1) * CB]
                )

    nc.compile()
    return nc


LAST_RESULTS = None


def _pack_core(positions, velocities, masses, n_tiles):
    """f32 [npc,3]x2 + [npc] -> bf16 [T, 128, 6G] planar (x y vx vy m vz)."""
    out = np.empty((n_tiles, P, 6 * G), dtype=ml_dtypes.bfloat16)
    pr = positions.reshape(n_tiles, P, G, 3)
    vr = velocities.reshape(n_tiles, P, G, 3)
    out[:, :, 0 * G : 1 * G] = pr[:, :, :, 0]
    out[:, :, 1 * G : 2 * G] = pr[:, :, :, 1]
    out[:, :, 2 * G : 3 * G] = vr[:, :, :, 0]
    out[:, :, 3 * G : 4 * G] = vr[:, :, :, 1]
    out[:, :, 4 * G : 5 * G] = masses.reshape(n_tiles, P, G)
    out[:, :, 5 * G : 6 * G] = vr[:, :, :, 2]
    return out


def _postprocess(hsum, n_quads):
    """hsum [P, CB] f64 (bank-reduced) -> kin [6, 50] f32."""
    # quadrant reduce: rows 32q + j, j in 0..13
    H = np.zeros((2 * F, CB), dtype=np.float64)
    for q in range(n_quads):
        H += hsum[32 * q : 32 * q + 2 * F, :]
    # H[(a*F + f), c] -> virtual bins idx = 2c + a
    nb = 2 * CB
    hist = np.zeros((nb, F), dtype=np.float64)
    for a in range(2):
        for c in range(CB):
            hist[2 * c + a, :] = H[a * F : (a + 1) * F, c]
    h = hist[:50]
    mass = h[:, 0]
    with np.errstate(divide="ignore", invalid="ignore"):
        mm = h[:, 1:] / mass[:, None]
        vr_m = 5.0 * mm[:, 2]     # sp plane (f index 3 in planes? see order)
        # feature order within f=1..6: [vz, vz2, sp, sp2, tp, tp2]
        vz_m = mm[:, 0]
        vz2 = mm[:, 1]
        vr_m = 5.0 * mm[:, 2]
        vr2 = 25.0 * mm[:, 3]
        vph_m = 5.0 * mm[:, 4]
        vph2 = 25.0 * mm[:, 5]
        vr_sig = np.sqrt(np.maximum(vr2 - vr_m**2, 0.0))
        vph_sig = np.sqrt(np.maximum(vph2 - vph_m**2, 0.0))
        vz_sig = np.sqrt(np.maximum(vz2 - vz_m**2, 0.0))
    return np.stack((vph_m, vph_sig, vr_m, vr_sig, vz_m, vz_sig)).astype(np.float32)


N_QUADS = 1
N_BANKS = 1


def kernel(positions, velocities, masses, trace=False):
    global LAST_RESULTS
    positions = np.ascontiguousarray(np.asarray(positions, dtype=np.float32))
    velocities = np.ascontiguousarray(np.asarray(velocities, dtype=np.float32))
    masses = np.ascontiguousarray(np.asarray(masses, dtype=np.float32))
    n = positions.shape[0]
    assert n % (N_CORES * P * G) == 0, n
    npc = n // N_CORES
    n_tiles = npc // (P * G)

    key = (n_tiles, N_QUADS, N_BANKS)
    if key not in _CACHE:
        _CACHE[key] = _build(n_tiles, n_quads=N_QUADS, n_banks=N_BANKS)
    nc = _CACHE[key]

    in_maps = []
    for k in range(N_CORES):
        sl = slice(k * npc, (k + 1) * npc)
        in_maps.append(
            {"data": _pack_core(positions[sl], velocities[sl], masses[sl], n_tiles)}
        )

    res = run_bass_kernel_spmd(nc, in_maps, core_ids=list(range(N_CORES)), trace=trace)
    LAST_RESULTS = res

    hsum = np.zeros((P, CB), dtype=np.float64)
    for r in res.results:
        hsum += r["hist"].astype(np.float64).sum(axis=0)
    return _postprocess(hsum, N_QUADS)
